# revision 20
# baseline (speedup 1.0000x reference)
"""Data-adaptive weight-ensembling MLP (per-sample expert-merged FFN) on 8 trn2 cores.

Math (per sample b):
  c[b,:,:]  = gate(x)[b].reshape(E, L)          (2-layer relu MLP gate)
  W1[b] = bW1 + sum_e c[b,e,0] tvW1[e];  b1[b] = bb1 + sum_e c[b,e,1] tvb1[e]
  W2[b] = bW2 + sum_e c[b,e,2] tvW2[e];  b2[b] = bb2 + sum_e c[b,e,3] tvb2[e]
  out[b] = relu(x[b] @ W1[b].T + b1[b]) @ W2[b].T + b2[b]

Merged weights are never materialized:
  x[b] @ W1[b].T = x[b] @ bW1.T + sum_e c[b,e,0] (x[b] @ tvW1[e].T)
and the weighted expert sum happens inside PSUM accumulation: for expert e the
matmul stationary operand is X1T[e][d, b] = x[b, d] * c[b, e, 0], so every
task-vector element streams through the PE exactly once.

fp8 cfg: task-vector banks and gate layer-1 weights are stored in e4m3
(pre-scaled); the stationary x*c stays bf16 (mixed-dtype matmul). A per-layer
residual stream R = sum_e (tv_e - q(tv_e)) rides along as an 18th matmul with
coefficient mean_e c[b,e], cancelling the common-mode quantization error.
All layer matmuls (bias/base/experts) accumulate into one PSUM at scale SW;
the final activation rescales by 1/SW.

Sharding (8 cores): DFF=4096 split into 8 slices of 512. Core k computes
layer-1 output columns in its slice, then contracts layer 2 over the same
f-slice. Layer 2 runs in two column chunks (256 then 768): each chunk's
partial output is ReduceScattered across the 8 cores, with the first RS
hidden under the second chunk's matmuls (a tiny warmup AllReduce at kernel
start absorbs the cc-stream cold cost). The host concatenates the per-core
row slices. Gate layer 1 runs as e4m3 DoubleRow matmuls.
"""

import contextlib

import numpy as np

B, D, DFF, E, L = 16, 1024, 4096, 16, 4
NCORES = 8
OSL = DFF // NCORES          # 512: per-core DFF slice
KC1 = D // 128               # 8 k-chunks for the d contraction
KC2 = OSL // 128             # 4 k-chunks for the f contraction
NE = E + 1                   # experts + residual-correction stream
SW1 = 512.0                  # fp8 scale for tv1 (tv std 0.02 -> ~10)
SW2 = 512.0                  # fp8 scale for tv2
SG = 2048.0                  # fp8 scale for gate W1 (std 0.01 -> ~20)
SXG = 16.0                   # fp8 scale for the gate stationary x
RMULT = 8.0                  # extra scale on the R streams
PRE2 = 4                     # tv2 half-tiles pre-issued before the L1 loop

_cache = {}


def _build(reps: int = 1, collective: bool = True, cfg: str = "fp8"):
    import concourse.bacc as bacc
    import concourse.bass as bass
    import concourse.tile as tile
    import concourse.mybir as mybir
    from concourse.masks import make_identity

    f32 = mybir.dt.float32
    bf = mybir.dt.bfloat16
    f8 = mybir.dt.float8e4
    f32r = mybir.dt.float32r
    fp8 = cfg == "fp8"
    if cfg in ("bf16", "fp8"):
        wdt = bf
        mmcast = lambda ap: ap
    elif cfg == "f32r":
        wdt = f32
        mmcast = lambda ap: ap.bitcast(f32r)
    else:
        wdt = f32
        mmcast = lambda ap: ap
    tvdt = f8 if fp8 else wdt
    gdt = f8 if fp8 else wdt
    nexp = NE if fp8 else E
    Relu = mybir.ActivationFunctionType.Relu
    Copy = mybir.ActivationFunctionType.Copy
    nc = bacc.Bacc("TRN2", target_bir_lowering=False, debug=False,
                   num_devices=NCORES)

    # ---- I/O (per-core data supplied via in_maps) ----
    xT_h = nc.dram_tensor("xT", [128, KC1, B], wdt, kind="ExternalInput")
    gw1_h = nc.dram_tensor("gw1", [128, KC1, D], gdt, kind="ExternalInput")
    gb1_h = nc.dram_tensor("gb1v", [1, D], wdt, kind="ExternalInput")
    gw2_h = nc.dram_tensor("gw2", [128, KC1, E * L], wdt, kind="ExternalInput")
    gb2_h = nc.dram_tensor("gb2v", [1, E * L], f32, kind="ExternalInput")
    tv1_h = nc.dram_tensor("tv1", [nexp, 128, KC1, OSL], tvdt,
                           kind="ExternalInput")
    bw1_h = nc.dram_tensor("bw1", [128, KC1, OSL], wdt, kind="ExternalInput")
    bb1_h = nc.dram_tensor("bb1v", [1, OSL], wdt, kind="ExternalInput")
    tvb1_h = nc.dram_tensor("tvb1", [E, OSL], wdt, kind="ExternalInput")
    # layer-2 task vectors stored chunk-major: cols [0:256] and [256:1024]
    if fp8:
        tv2a_h = nc.dram_tensor("tv2a", [nexp, 128, KC2, 256], tvdt,
                                kind="ExternalInput")
        tv2d_h = nc.dram_tensor("tv2d", [nexp, 128, KC2, 768], tvdt,
                                kind="ExternalInput")
    else:
        tv2_h = nc.dram_tensor("tv2", [nexp, 2, 128, KC2, 512], tvdt,
                               kind="ExternalInput")
    bw2_h = nc.dram_tensor("bw2", [128, KC2, D], wdt, kind="ExternalInput")
    bb2_h = nc.dram_tensor("bb2v", [1, D], wdt, kind="ExternalInput")
    tvb2_h = nc.dram_tensor("tvb2", [E, D], wdt, kind="ExternalInput")
    if fp8:
        CW = [256, 768]
        out_h = [nc.dram_tensor(f"out{n}", [B // NCORES, CW[n]], f32,
                                kind="ExternalOutput") for n in range(2)]
        ar_in = [nc.dram_tensor(f"ar_in{n}", [B, CW[n]], f32,
                                kind="Internal") for n in range(2)]
        ar_out = [nc.dram_tensor(f"ar_out{n}", [B // NCORES, CW[n]], f32,
                                 kind="Internal") for n in range(2)]
        wu_in = nc.dram_tensor("wu_in", [1, B], f32, kind="Internal")
        wu_out = nc.dram_tensor("wu_out", [1, B], f32, kind="Internal",
                                addr_space="Shared")
    else:
        out_full = nc.dram_tensor("out", [B, D], f32, kind="ExternalOutput")
        ar_in_full = nc.dram_tensor("ar_in", [B, D], f32, kind="Internal")
        ar_out_full = nc.dram_tensor("ar_out", [B, D], f32, kind="Internal",
                                     addr_space="Shared")

    with tile.TileContext(nc) as tc, contextlib.ExitStack() as ctx:
        const = ctx.enter_context(tc.tile_pool(name="const", bufs=1))
        small = ctx.enter_context(tc.tile_pool(name="small", bufs=1))
        gwp = ctx.enter_context(tc.tile_pool(name="gwp", bufs=1))
        basep = ctx.enter_context(tc.tile_pool(name="basep", bufs=1))
        tvp1 = ctx.enter_context(tc.tile_pool(name="tvp1", bufs=9))
        tvp2a = ctx.enter_context(tc.tile_pool(name="tvp2a", bufs=4))
        tvp2d = ctx.enter_context(tc.tile_pool(name="tvp2d", bufs=4))
        pacc = ctx.enter_context(tc.tile_pool(name="pacc", bufs=1,
                                              space="PSUM"))
        psml = ctx.enter_context(tc.tile_pool(name="psml", bufs=2,
                                              space="PSUM"))

        # constants (once)
        ones1 = const.tile([1, B], f32)
        nc.vector.memset(ones1[:], 1.0)
        ones1b = const.tile([1, B], wdt)
        nc.vector.memset(ones1b[:], 1.0)
        ident16 = const.tile([B, B], f32)
        make_identity(nc, ident16[:])
        ones16_128 = const.tile([B, 128], wdt)
        nc.vector.memset(ones16_128[:], 1.0)
        if fp8:
            # identity / (E * RMULT): builds the R-stream coefficient
            # (mean_e c[b,e]) / RMULT from a row-sum of codings
            identR = const.tile([B, B], f32)
            make_identity(nc, identR[:])
            nc.vector.tensor_scalar_mul(identR[:], identR[:],
                                        1.0 / (E * RMULT))
            # layer-2 coefficients carry 1/SW2 so psum2 is true-valued
            identS2 = const.tile([B, B], f32)
            make_identity(nc, identS2[:])
            nc.vector.tensor_scalar_mul(identS2[:], identS2[:], 1.0 / SW2)
            identR2 = const.tile([B, B], f32)
            make_identity(nc, identR2[:])
            nc.vector.tensor_scalar_mul(identR2[:], identR2[:],
                                        1.0 / (E * RMULT * SW2))

        for _rep in range(reps):
            if fp8 and collective:
                # tiny early collective to absorb the cold-start cost of the
                # cc stream while compute runs
                wu = small.tile([1, B], f32, name=f"wu_{_rep}", tag="wu")
                nc.vector.memset(wu[:], 0.0)
                nc.gpsimd.dma_start(out=wu_in.ap(), in_=wu[:])
                nc.gpsimd.collective_compute(
                    "AllReduce", mybir.AluOpType.add,
                    replica_groups=[list(range(NCORES))],
                    ins=[wu_in.ap().opt()],
                    outs=[wu_out.ap().opt()],
                )
            # small inputs
            xT = small.tile([128, KC1, B], wdt, name=f"xT_{_rep}", tag="xT")
            nc.sync.dma_start(out=xT[:], in_=xT_h.ap())
            gb1v = small.tile([1, D], wdt, name=f"gb1v_{_rep}", tag="gb1v")
            nc.sync.dma_start(out=gb1v[:], in_=gb1_h.ap())
            gb2v = small.tile([1, E * L], f32, name=f"gb2v_{_rep}", tag="gb2v")
            nc.sync.dma_start(out=gb2v[:], in_=gb2_h.ap())
            gw1a = gwp.tile([128, KC1 // 2, D], gdt, name=f"gw1a_{_rep}",
                            tag="gw1a")
            nc.sync.dma_start(out=gw1a[:], in_=gw1_h.ap()[:, 0:KC1 // 2, :])
            gw1b = gwp.tile([128, KC1 // 2, D], gdt, name=f"gw1b_{_rep}",
                            tag="gw1b")
            nc.sync.dma_start(out=gw1b[:], in_=gw1_h.ap()[:, KC1 // 2:KC1, :])
            gw2t = small.tile([128, KC1, E * L], wdt, name=f"gw2t_{_rep}",
                              tag="gw2t")
            nc.sync.dma_start(out=gw2t[:], in_=gw2_h.ap())
            bb1v = small.tile([1, OSL], wdt, name=f"bb1v_{_rep}", tag="bb1v")
            nc.sync.dma_start(out=bb1v[:], in_=bb1_h.ap())
            tvb1t = small.tile([E, OSL], wdt, name=f"tvb1t_{_rep}", tag="tvb1t")
            nc.sync.dma_start(out=tvb1t[:], in_=tvb1_h.ap())
            bb2v = small.tile([1, D], wdt, name=f"bb2v_{_rep}", tag="bb2v")
            nc.sync.dma_start(out=bb2v[:], in_=bb2_h.ap())
            tvb2t = small.tile([E, D], wdt, name=f"tvb2t_{_rep}", tag="tvb2t")
            nc.sync.dma_start(out=tvb2t[:], in_=tvb2_h.ap())
            # base weights prefetch (consumed early in each layer)
            base1 = basep.tile([128, KC1, OSL], wdt, name=f"base1_{_rep}",
                               tag="base1")
            nc.sync.dma_start(out=base1[:], in_=bw1_h.ap())
            base2 = basep.tile([128, KC2, D], wdt, name=f"base2_{_rep}",
                               tag="base2")
            nc.sync.dma_start(out=base2[:], in_=bw2_h.ap())

            # ---- L1 bias + base matmuls first (independent of the gate) ----
            psum1 = pacc.tile([B, OSL], f32, tag="psum1")
            nc.tensor.matmul(psum1[:], ones1b[:], bb1v[:], start=True,
                             stop=False)
            for kc in range(KC1):
                nc.tensor.matmul(psum1[:], mmcast(xT[:, kc, :]),
                                 mmcast(base1[:, kc, :]),
                                 start=False, stop=False)

            # ---- gate layer 1: g_h = relu((x @ (gW1*SG).T + gb1*SG)/SG) ----
            g_h = small.tile([B, D], f32, name=f"g_h_{_rep}", tag="g_h")
            if fp8:
                # e4m3 copy of x (*SXG) for DoubleRow gate matmuls
                x8 = small.tile([128, KC1, B], f8, name=f"x8_{_rep}", tag="x8")
                nc.vector.tensor_scalar_mul(x8[:], xT[:], SXG)
                DR = mybir.MatmulPerfMode.DoubleRow
                for n in range(2):
                    gps = pacc.tile([B, 512], f32, tag="gps")
                    nc.tensor.matmul(gps[:], ones1b[:],
                                     gb1v[:, n * 512:(n + 1) * 512],
                                     start=True, stop=False)
                    for kc in range(0, KC1, 2):
                        gwt = gw1a if kc < KC1 // 2 else gw1b
                        ko = kc if kc < KC1 // 2 else kc - KC1 // 2
                        nc.tensor.matmul(
                            gps[:], x8[:, kc:kc + 2, :],
                            gwt[:, ko:ko + 2, n * 512:(n + 1) * 512],
                            start=False, stop=(kc == KC1 - 2),
                            perf_mode=DR)
                    nc.scalar.activation(g_h[:, n * 512:(n + 1) * 512],
                                         gps[:], Relu, scale=1.0 / (SG * SXG))
            else:
                for n in range(2):
                    gps = pacc.tile([B, 512], f32, tag="gps")
                    nc.tensor.matmul(gps[:], ones1b[:],
                                     gb1v[:, n * 512:(n + 1) * 512],
                                     start=True, stop=False)
                    for kc in range(KC1):
                        gwt = gw1a if kc < KC1 // 2 else gw1b
                        ko = kc if kc < KC1 // 2 else kc - KC1 // 2
                        nc.tensor.matmul(
                            gps[:], mmcast(xT[:, kc, :]),
                            mmcast(gwt[:, ko, n * 512:(n + 1) * 512]),
                            start=False, stop=(kc == KC1 - 1))
                    nc.scalar.activation(g_h[:, n * 512:(n + 1) * 512],
                                         gps[:], Relu)

            # ---- transpose g_h -> ghT [128, (kc, b)] ----
            ghT = small.tile([128, KC1, B], wdt, name=f"ghT_{_rep}", tag="ghT")
            for kc in range(KC1):
                pt = psml.tile([128, B], f32, tag="ps")
                nc.tensor.transpose(pt[:], g_h[:, kc * 128:(kc + 1) * 128],
                                    ident16[:])
                nc.vector.tensor_copy(ghT[:, kc, :], pt[:])

            # ---- gate layer 2: codings; cod[b, e, l] ----
            cps = psml.tile([B, E * L], f32, tag="ps")
            nc.tensor.matmul(cps[:], ones1[:], gb2v[:], start=True, stop=False)
            for kc in range(KC1):
                nc.tensor.matmul(cps[:], mmcast(ghT[:, kc, :]),
                                 mmcast(gw2t[:, kc, :]),
                                 start=False, stop=(kc == KC1 - 1))
            cod = small.tile([B, E, L], f32, name=f"cod_{_rep}", tag="cod")
            nc.vector.tensor_copy(cod[:],
                                  cps[:].rearrange("b (e l) -> b e l", e=E))

            # ---- bias-coefficient matrices cT_l[e, b] = c[b, e, l] ----
            cT = {}
            for l in (1, 3):
                cl = small.tile([B, E], f32, name=f"cl{l}_{_rep}",
                                tag=f"cl{l}")
                nc.vector.tensor_copy(cl[:], cod[:, :, l])
                ptc = psml.tile([B, E], f32, tag="ps")
                nc.tensor.transpose(ptc[:], cl[:], ident16[:])
                cTl = small.tile([E, B], wdt, name=f"cT{l}_{_rep}",
                                 tag=f"cT{l}")
                nc.vector.tensor_copy(cTl[:], ptc[:])
                cT[l] = cTl

            # ---- per-expert coefficients, incl. R coefficient ----
            cmean = {}
            if fp8:
                for l in (0, 2):
                    cm = small.tile([B, 1], f32, name=f"cm{l}_{_rep}",
                                    tag=f"cm{l}")
                    nc.vector.tensor_reduce(cm[:], cod[:, :, l],
                                            axis=mybir.AxisListType.X,
                                            op=mybir.AluOpType.add)
                    cmean[l] = cm

            # ---- broadcast tiles cbc[l][e][p, b] = coeff[b, e] ----
            cbc = {0: [], 2: []}
            for l in (0, 2):
                for e in range(nexp):
                    diag = small.tile([B, B], wdt, name=f"dg{l}_{e}_{_rep}",
                                      tag="diag")
                    if fp8 and l == 2:
                        idT = identR2 if e == E else identS2
                    elif fp8 and e == E:
                        idT = identR
                    else:
                        idT = ident16
                    nc.vector.tensor_scalar_mul(
                        diag[:], idT[:],
                        cmean[l][:, 0:1] if (fp8 and e == E)
                        else cod[:, e, l:l + 1])
                    pb = psml.tile([128, B], f32, tag="ps")
                    nc.tensor.matmul(pb[:], ones16_128[:], diag[:],
                                     start=True, stop=True)
                    bc = small.tile([128, B], wdt, name=f"bc{l}_{e}_{_rep}",
                                    tag=f"bc{l}_{e}")
                    nc.vector.tensor_copy(bc[:], pb[:])
                    cbc[l].append(bc)

            # ---- X1T[e][128, kc, b] = xT * c1[b, e] ----
            x1t = []
            for e in range(nexp):
                t = small.tile([128, KC1, B], wdt, name=f"x1t{e}_{_rep}",
                               tag=f"x1t{e}")
                nc.vector.tensor_mul(
                    t[:], xT[:],
                    cbc[0][e][:, None, :].broadcast_to([128, KC1, B]))
                x1t.append(t)

            # pre-issue the first tv2 tiles (both chunks) so their DMA
            # runs during L1 and the L1->L2 transition never starves DMA
            tv2_tiles = {}
            if fp8:
                for e in range(PRE2):
                    t = tvp2a.tile([128, KC2, 256], tvdt, tag="tvt2a")
                    nc.sync.dma_start(out=t[:], in_=tv2a_h.ap()[e])
                    tv2_tiles[(0, e)] = t
                for e in range(2):
                    t = tvp2d.tile([128, KC2, 768], tvdt, tag="tvt2d")
                    nc.sync.dma_start(out=t[:], in_=tv2d_h.ap()[e])
                    tv2_tiles[(1, e)] = t

            # ---- layer 1 experts (base/bias already accumulated) ----
            nc.tensor.matmul(psum1[:], cT[1][:], tvb1t[:],
                             start=False, stop=False)
            for e in range(nexp):
                tvt = tvp1.tile([128, KC1, OSL], tvdt, tag="tvt1")
                nc.sync.dma_start(out=tvt[:], in_=tv1_h.ap()[e])
                for kc in range(KC1):
                    nc.tensor.matmul(psum1[:], mmcast(x1t[e][:, kc, :]),
                                     mmcast(tvt[:, kc, :]),
                                     start=False,
                                     stop=(e == nexp - 1 and kc == KC1 - 1))

            h1 = small.tile([B, OSL], f32, name=f"h1_{_rep}", tag="h1")
            nc.scalar.activation(h1[:], psum1[:], Relu,
                                 scale=(1.0 / SW1) if fp8 else 1.0)

            # ---- transpose h1 -> h1T [128, (fc, b)] ----
            h1T = small.tile([128, KC2, B], wdt, name=f"h1T_{_rep}", tag="h1T")
            for fc in range(KC2):
                pt2 = psml.tile([128, B], f32, tag="ps")
                nc.tensor.transpose(pt2[:], h1[:, fc * 128:(fc + 1) * 128],
                                    ident16[:])
                nc.vector.tensor_copy(h1T[:, fc, :], pt2[:])

            # ---- X2T[e][128, fc, b] = h1T * c2[b, e] ----
            x2t = []
            for e in range(nexp):
                t = small.tile([128, KC2, B], wdt, name=f"x2t{e}_{_rep}",
                               tag=f"x2t{e}")
                nc.vector.tensor_mul(
                    t[:], h1T[:],
                    cbc[2][e][:, None, :].broadcast_to([128, KC2, B]))
                x2t.append(t)

            # ---- layer 2 in two column halves; overlap RS of half 0 ----
            if fp8:
                # chunk A: cols 0:256 -> early RS0
                pA = pacc.tile([B, 256], f32, tag="psum2_a")
                nc.tensor.matmul(pA[:], ones1b[:], bb2v[:, 0:256],
                                 start=True, stop=False)
                nc.tensor.matmul(pA[:], cT[3][:], tvb2t[:, 0:256],
                                 start=False, stop=False)
                for fc in range(KC2):
                    nc.tensor.matmul(pA[:], mmcast(h1T[:, fc, :]),
                                     mmcast(base2[:, fc, 0:256]),
                                     start=False, stop=False)
                for e in range(nexp):
                    t = tv2_tiles.pop((0, e), None)
                    if t is None:
                        t = tvp2a.tile([128, KC2, 256], tvdt, tag="tvt2a")
                        nc.sync.dma_start(out=t[:], in_=tv2a_h.ap()[e])
                    for fc in range(KC2):
                        nc.tensor.matmul(pA[:], mmcast(x2t[e][:, fc, :]),
                                         mmcast(t[:, fc, :]),
                                         start=False,
                                         stop=(e == nexp - 1
                                               and fc == KC2 - 1))
                oA = small.tile([B, 256], f32, name=f"oA_{_rep}", tag="oA")
                nc.scalar.activation(oA[:], pA[:], Copy)
                if collective:
                    nc.scalar.dma_start(out=ar_in[0].ap(), in_=oA[:])
                    nc.gpsimd.collective_compute(
                        "ReduceScatter", mybir.AluOpType.add,
                        replica_groups=[list(range(NCORES))],
                        ins=[ar_in[0].ap().opt()],
                        outs=[ar_out[0].ap().opt()],
                    )
                    nc.scalar.dma_start(out=out_h[0].ap(),
                                        in_=ar_out[0].ap())
                else:
                    nc.sync.dma_start(out=out_h[0].ap(),
                                      in_=oA[0:B // NCORES, :])

                # chunk D: cols 256:1024 (psums B 512 + C 256) -> RS1
                pB = pacc.tile([B, 512], f32, tag="psum2_b")
                pC = pacc.tile([B, 256], f32, tag="psum2_c")
                nc.tensor.matmul(pB[:], ones1b[:], bb2v[:, 256:768],
                                 start=True, stop=False)
                nc.tensor.matmul(pC[:], ones1b[:], bb2v[:, 768:1024],
                                 start=True, stop=False)
                nc.tensor.matmul(pB[:], cT[3][:], tvb2t[:, 256:768],
                                 start=False, stop=False)
                nc.tensor.matmul(pC[:], cT[3][:], tvb2t[:, 768:1024],
                                 start=False, stop=False)
                for fc in range(KC2):
                    nc.tensor.matmul(pB[:], mmcast(h1T[:, fc, :]),
                                     mmcast(base2[:, fc, 256:768]),
                                     start=False, stop=False)
                    nc.tensor.matmul(pC[:], mmcast(h1T[:, fc, :]),
                                     mmcast(base2[:, fc, 768:1024]),
                                     start=False, stop=False)
                for e in range(nexp):
                    t = tv2_tiles.pop((1, e), None)
                    if t is None:
                        t = tvp2d.tile([128, KC2, 768], tvdt, tag="tvt2d")
                        nc.sync.dma_start(out=t[:], in_=tv2d_h.ap()[e])
                    last = e == nexp - 1
                    for fc in range(KC2):
                        nc.tensor.matmul(pB[:], mmcast(x2t[e][:, fc, :]),
                                         mmcast(t[:, fc, 0:512]),
                                         start=False,
                                         stop=(last and fc == KC2 - 1))
                        nc.tensor.matmul(pC[:], mmcast(x2t[e][:, fc, :]),
                                         mmcast(t[:, fc, 512:768]),
                                         start=False,
                                         stop=(last and fc == KC2 - 1))
                oD = small.tile([B, 768], f32, name=f"oD_{_rep}", tag="oD")
                nc.scalar.activation(oD[:, 0:512], pB[:], Copy)
                nc.scalar.activation(oD[:, 512:768], pC[:], Copy)
                if collective:
                    nc.scalar.dma_start(out=ar_in[1].ap(), in_=oD[:])
                    nc.gpsimd.collective_compute(
                        "ReduceScatter", mybir.AluOpType.add,
                        replica_groups=[list(range(NCORES))],
                        ins=[ar_in[1].ap().opt()],
                        outs=[ar_out[1].ap().opt()],
                    )
                    nc.scalar.dma_start(out=out_h[1].ap(),
                                        in_=ar_out[1].ap())
                else:
                    nc.sync.dma_start(out=out_h[1].ap(),
                                      in_=oD[0:B // NCORES, :])
            else:
                outp = small.tile([B, D], f32, name=f"outp_{_rep}", tag="outp")
                for n in range(2):
                    p = pacc.tile([B, 512], f32, tag=f"psum2_{n}")
                    nc.tensor.matmul(p[:], ones1b[:],
                                     bb2v[:, n * 512:(n + 1) * 512],
                                     start=True, stop=False)
                    nc.tensor.matmul(p[:], cT[3][:],
                                     tvb2t[:, n * 512:(n + 1) * 512],
                                     start=False, stop=False)
                    for fc in range(KC2):
                        nc.tensor.matmul(
                            p[:], mmcast(h1T[:, fc, :]),
                            mmcast(base2[:, fc, n * 512:(n + 1) * 512]),
                            start=False, stop=False)
                    for e in range(nexp):
                        tvt2 = tvp2.tile([128, KC2, 512], tvdt, tag="tvt2")
                        nc.sync.dma_start(out=tvt2[:], in_=tv2_h.ap()[e, n])
                        for fc in range(KC2):
                            nc.tensor.matmul(p[:], mmcast(x2t[e][:, fc, :]),
                                             mmcast(tvt2[:, fc, :]),
                                             start=False,
                                             stop=(e == nexp - 1
                                                   and fc == KC2 - 1))
                    nc.scalar.activation(outp[:, n * 512:(n + 1) * 512],
                                         p[:], Copy)
                if collective:
                    nc.sync.dma_start(out=ar_in_full.ap(), in_=outp[:])
                    nc.gpsimd.collective_compute(
                        "AllReduce", mybir.AluOpType.add,
                        replica_groups=[list(range(NCORES))],
                        ins=[ar_in_full.ap().opt()],
                        outs=[ar_out_full.ap().opt()],
                    )
                    nc.sync.dma_start(out=out_full.ap(), in_=ar_out_full.ap())
                else:
                    nc.sync.dma_start(out=out_full.ap(), in_=outp[:])

    nc.compile()
    return nc


def _prep_inputs(x, gW1, gb1, gW2, gb2, bW1, bb1, bW2, bb2,
                 tvW1, tvb1, tvW2, tvb2, cfg="fp8"):
    """Build the 8 per-core in_maps (DMA-friendly layouts)."""
    import ml_dtypes

    f = np.float32
    fp8 = cfg == "fp8"
    if cfg in ("bf16", "fp8"):
        w = np.dtype(ml_dtypes.bfloat16)
    else:
        w = f
    e4 = np.dtype(ml_dtypes.float8_e4m3)
    asf = lambda a: np.ascontiguousarray(a, dtype=f)
    asw = lambda a: np.ascontiguousarray(a.astype(f), dtype=w)
    q8 = lambda a: np.clip(a, -240.0, 240.0).astype(e4)

    xT = asw(x.T.reshape(KC1, 128, B).transpose(1, 0, 2))
    if fp8:
        gw1 = np.ascontiguousarray(
            q8(gW1.astype(f).T * SG).reshape(KC1, 128, D).transpose(1, 0, 2))
        gb1v = asw(gb1.reshape(1, D) * SG * SXG)
    else:
        gw1 = asw(gW1.T.reshape(KC1, 128, D).transpose(1, 0, 2))
        gb1v = asw(gb1.reshape(1, D))
    gw2 = asw(gW2.T.reshape(KC1, 128, E * L).transpose(1, 0, 2))
    gb2v = asf(gb2.reshape(1, E * L))

    s1 = SW1 if fp8 else 1.0
    s2 = SW2 if fp8 else 1.0

    in_maps = []
    for k in range(NCORES):
        o0 = k * OSL
        tv1s = tvW1[:, o0:o0 + OSL, :].astype(f) * s1     # [E, OSL, D]
        tv2s = tvW2[:, :, o0:o0 + OSL].astype(f) * s2     # [E, D, OSL]
        if fp8:
            tq1 = q8(tv1s)
            r1 = (tv1s.sum(0) - tq1.astype(f).sum(0)) * RMULT
            tv1all = np.concatenate([tq1, q8(r1)[None]], axis=0)
            tq2 = q8(tv2s)
            r2 = (tv2s.sum(0) - tq2.astype(f).sum(0)) * RMULT
            tv2all = np.concatenate([tq2, q8(r2)[None]], axis=0)
            tv1 = np.ascontiguousarray(
                tv1all.transpose(0, 2, 1)
                .reshape(NE, KC1, 128, OSL).transpose(0, 2, 1, 3))
            # [NE, OSL(f), D(j)] -> [NE, 128(f_p), KC2, D(j)] -> col chunks
            t2 = (tv2all.transpose(0, 2, 1).reshape(NE, KC2, 128, D)
                  .transpose(0, 2, 1, 3))
            tv2a = np.ascontiguousarray(t2[:, :, :, 0:256])
            tv2d = np.ascontiguousarray(t2[:, :, :, 256:1024])
        else:
            tv1 = asw(tv1s.transpose(0, 2, 1)
                      .reshape(E, KC1, 128, OSL).transpose(0, 2, 1, 3))
            t2 = tv2s.transpose(0, 2, 1).reshape(E, KC2, 128, 2, 512)
            tv2 = asw(t2.transpose(0, 3, 2, 1, 4))
        bw1 = asw((bW1[o0:o0 + OSL, :].astype(f) * s1).T
                  .reshape(KC1, 128, OSL).transpose(1, 0, 2))
        bw2 = asw((bW2[:, o0:o0 + OSL].astype(f) * (1.0 if fp8 else s2)).T
                  .reshape(KC2, 128, D).transpose(1, 0, 2))
        zero = k != 0
        in_maps.append(dict(
            xT=xT, gw1=gw1, gb1v=gb1v, gw2=gw2, gb2v=gb2v,
            tv1=tv1, bw1=bw1,
            bb1v=asw(bb1[o0:o0 + OSL].reshape(1, OSL) * s1),
            tvb1=asw(tvb1[:, o0:o0 + OSL] * s1),
            bw2=bw2,
            bb2v=np.zeros((1, D), w) if zero
            else asw(bb2.reshape(1, D) * (1.0 if fp8 else s2)),
            tvb2=np.zeros((E, D), w) if zero
            else asw(tvb2 * (1.0 if fp8 else s2)),
            **(dict(tv2a=tv2a, tv2d=tv2d) if fp8 else dict(tv2=tv2)),
        ))
    return in_maps


CFG = "fp8"


def kernel(**inputs):
    from concourse.bass_utils import run_bass_kernel_spmd

    key = ("nc", CFG)
    if key not in _cache:
        _cache[key] = _build(cfg=CFG)
    nc = _cache[key]

    in_maps = _prep_inputs(**{k: np.asarray(v) for k, v in inputs.items()},
                           cfg=CFG)
    res = run_bass_kernel_spmd(nc, in_maps, core_ids=list(range(NCORES)))
    if CFG == "fp8":
        out = np.empty((B, D), np.float32)
        rows = B // NCORES
        for k in range(NCORES):
            out[k * rows:(k + 1) * rows, 0:256] = res.results[k]["out0"]
            out[k * rows:(k + 1) * rows, 256:1024] = res.results[k]["out1"]
        return out
    return res.results[0]["out"]
